# revision 13
# baseline (speedup 1.0000x reference)
"""Trainium2 Bass kernel for nn_AggFeatureSeqEncoder (histogram binning).

Per row b: row stats over T, plus per-category (V=256 and V=128)
count / value-sum / value^2-sum histograms, reduced to
count / mean / std features, plus distinct-category counts.

Sharding: pure data parallelism, B=4096 rows split 512/core over 8 cores.
"""

import numpy as np

import concourse.bass as bass
import concourse.bacc as bacc
import concourse.mybir as mybir
from concourse.tile import TileContext
from concourse.bass_utils import run_bass_kernel_spmd

B, T = 4096, 2048
NCORES = 8
R = B // NCORES  # rows per core
PT = 128         # partition tile (rows per SBUF tile)
VM, VT = 256, 128
H = 4 + 3 * VM + 3 * VT + 2  # 1158
EPS = 1e-9

f32 = mybir.dt.float32
bf16 = mybir.dt.bfloat16
i32 = mybir.dt.int32
Alu = mybir.AluOpType
Act = mybir.ActivationFunctionType


def _emit_tile(nc, ld, pool, hpool, amount, mcc, tr, sl_sb, out, it, t_len):
    """Emit instructions for one 128-row tile."""
    rows = slice(it * PT, (it + 1) * PT)

    # ---- loads -----------------------------------------------------------
    a = ld.tile([PT, t_len], f32, tag="a")
    nc.gpsimd.dma_start(out=a[:], in_=amount[rows, :])
    mcc_t = ld.tile([PT, t_len], i32, tag="mcc")
    nc.gpsimd.dma_start(out=mcc_t[:], in_=mcc[rows, :])
    tr_t = ld.tile([PT, t_len], i32, tag="tr")
    nc.gpsimd.dma_start(out=tr_t[:], in_=tr[rows, :])

    # ---- val = sign(a) * (exp(|a|) - 1) ---------------------------------
    sgn = pool.tile([PT, t_len], f32, tag="sgn")
    nc.scalar.activation(sgn[:], a[:], Act.Sign)
    ex = pool.tile([PT, t_len], f32, tag="ex")
    nc.scalar.activation(ex[:], a[:], Act.Abs)
    nc.scalar.activation(ex[:], ex[:], Act.Exp)
    val = pool.tile([PT, t_len], f32, tag="val")
    # val = (ex - 1) * sgn
    nc.vector.scalar_tensor_tensor(val[:], ex[:], 1.0, sgn[:], Alu.subtract, Alu.mult)

    # ---- row stats -------------------------------------------------------
    sum_ = hpool.tile([PT, 1], f32, tag="sum")
    sumsq = hpool.tile([PT, 1], f32, tag="sumsq")
    scr = pool.tile([PT, t_len], f32, tag="scr")
    nc.scalar.activation(scr[:], val[:], Act.Copy, accum_out=sum_[:])
    # val2 must be the square of the SAME val used for sv, or the
    # sv2 - sv^2/cnt cancellation breaks for single-element bins.
    val2_f = pool.tile([PT, t_len], f32, tag="val2f")
    nc.scalar.activation(val2_f[:], val[:], Act.Square, accum_out=sumsq[:])

    # bf16 copies of the codes (exact: values < 256)
    mcc_b = pool.tile([PT, t_len], bf16, tag="mccb")
    nc.gpsimd.tensor_scalar_add(mcc_b[:], mcc_t[:], 0)
    tr_b = pool.tile([PT, t_len], bf16, tag="trb")
    nc.gpsimd.tensor_scalar_add(tr_b[:], tr_t[:], 0)

    out_sb = hpool.tile([PT, H], f32, tag="outsb")

    sl_f = hpool.tile([PT, 1], f32, tag="slf")
    nc.vector.tensor_scalar_add(sl_f[:], sl_sb[:, it : it + 1], 0)
    # mean = sum / (sl + eps)
    tmp0 = hpool.tile([PT, 1], f32, tag="tmp0")
    nc.vector.tensor_scalar_add(tmp0[:], sl_f[:], EPS)
    r1 = hpool.tile([PT, 1], f32, tag="r1")
    nc.vector.reciprocal(r1[:], tmp0[:])
    nc.vector.tensor_tensor(out_sb[:, 2:3], sum_[:], r1[:], Alu.mult)
    # std = sqrt(clip(sumsq - sum^2/(sl+eps), 0) / (clip(sl-1,0)+eps))
    s2 = hpool.tile([PT, 1], f32, tag="s2")
    nc.vector.tensor_tensor(s2[:], sum_[:], sum_[:], Alu.mult)
    nc.vector.tensor_tensor(s2[:], s2[:], r1[:], Alu.mult)
    av = hpool.tile([PT, 1], f32, tag="av")
    nc.vector.tensor_tensor(av[:], sumsq[:], s2[:], Alu.subtract)
    nc.vector.tensor_scalar_max(av[:], av[:], 0.0)
    d = hpool.tile([PT, 1], f32, tag="d")
    nc.vector.tensor_scalar_sub(d[:], sl_f[:], 1.0)
    nc.vector.tensor_scalar_max(d[:], d[:], 0.0)
    nc.vector.tensor_scalar_add(d[:], d[:], EPS)
    nc.vector.reciprocal(d[:], d[:])
    nc.vector.tensor_tensor(av[:], av[:], d[:], Alu.mult)
    nc.scalar.activation(out_sb[:, 3:4], av[:], Act.Sqrt)
    nc.vector.tensor_scalar_add(out_sb[:, 0:1], sl_f[:], 0.0)
    nc.vector.tensor_scalar_add(out_sb[:, 1:2], sum_[:], 0.0)

    # ---- histograms ------------------------------------------------------
    eq = pool.tile([PT, t_len], bf16, tag="eq")
    eq2 = pool.tile([PT, t_len], f32, tag="eq2")
    eq3 = pool.tile([PT, t_len], f32, tag="eq3")

    for name, code_b, V in (("m", mcc_b, VM), ("t", tr_b, VT)):
        cnt = hpool.tile([PT, V], f32, tag="cnt" + name)
        sv = hpool.tile([PT, V], f32, tag="sv" + name)
        sv2 = hpool.tile([PT, V], f32, tag="sv2" + name)
        nc.vector.memset(cnt[:, 0:1], 0.0)
        nc.vector.memset(sv[:, 0:1], 0.0)
        nc.vector.memset(sv2[:, 0:1], 0.0)
        for v in range(1, V):
            fv = float(v)
            nc.vector.tensor_scalar(
                eq[:], code_b[:], fv, None, Alu.is_equal, Alu.add,
                accum_out=cnt[:, v : v + 1],
            )
            nc.vector.scalar_tensor_tensor(
                eq2[:], code_b[:], fv, val[:], Alu.is_equal, Alu.mult,
                accum_out=sv[:, v : v + 1],
            )
            nc.vector.scalar_tensor_tensor(
                eq3[:], code_b[:], fv, val2_f[:], Alu.is_equal, Alu.mult,
                accum_out=sv2[:, v : v + 1],
            )

        # ---- per-category features --------------------------------------
        if name == "m":
            o_cnt, o_mean, o_std, o_dist = 4, 4 + VM, 4 + 2 * VM, H - 2
        else:
            base = 4 + 3 * VM
            o_cnt, o_mean, o_std, o_dist = base, base + VT, base + 2 * VT, H - 1

        ct = hpool.tile([PT, V], f32, tag="ct" + name)
        rc = hpool.tile([PT, V], f32, tag="rc" + name)
        nc.vector.tensor_scalar_add(ct[:], cnt[:], EPS)
        nc.vector.reciprocal(rc[:], ct[:])
        mean_c = out_sb[:, o_mean : o_mean + V]
        nc.vector.tensor_tensor(mean_c, sv[:], rc[:], Alu.mult)
        # av = clip(sv2 - sv*mean, 0); denom = clip(cnt-1,0)+eps
        q = hpool.tile([PT, V], f32, tag="q" + name)
        nc.vector.tensor_tensor(q[:], sv[:], mean_c, Alu.mult)
        nc.vector.tensor_tensor(q[:], sv2[:], q[:], Alu.subtract)
        nc.vector.tensor_scalar_max(q[:], q[:], 0.0)
        nc.vector.tensor_scalar_sub(ct[:], cnt[:], 1.0)
        nc.vector.tensor_scalar_max(ct[:], ct[:], 0.0)
        nc.vector.tensor_scalar_add(ct[:], ct[:], EPS)
        nc.vector.reciprocal(rc[:], ct[:])
        nc.vector.tensor_tensor(q[:], q[:], rc[:], Alu.mult)
        nc.scalar.activation(out_sb[:, o_std : o_std + V], q[:], Act.Sqrt)
        nc.vector.tensor_scalar_add(out_sb[:, o_cnt : o_cnt + V], cnt[:], 0.0)
        # distinct count
        nc.vector.tensor_scalar(
            ct[:], cnt[:], 0.0, None, Alu.is_gt, Alu.add,
            accum_out=out_sb[:, o_dist : o_dist + 1],
        )

    out_stage = hpool.tile([PT, H], f32, tag="outstage")
    nc.scalar.activation(out_stage[:], out_sb[:], Act.Copy)
    nc.gpsimd.dma_start(out=out[rows, :], in_=out_stage[:])


def build_nc(r=R, t_len=T):
    nc = bacc.Bacc()
    amount = nc.declare_dram_parameter("amount", [r, t_len], f32, False)
    mcc = nc.declare_dram_parameter("mcc", [r, t_len], i32, False)
    tr = nc.declare_dram_parameter("tr", [r, t_len], i32, False)
    seq = nc.declare_dram_parameter("seq", [r, 1], i32, False)
    out = nc.declare_dram_parameter("out", [r, H], f32, True)
    with TileContext(nc) as tc:
        with (
            tc.tile_pool(name="ld", bufs=2) as ld,
            tc.tile_pool(name="wk", bufs=1) as pool,
            tc.tile_pool(name="hist", bufs=1) as hpool,
        ):
            nt = r // PT
            sl_sb = hpool.tile([PT, nt], i32, tag="slsb")
            nc.gpsimd.dma_start(
                out=sl_sb[:],
                in_=seq[:].rearrange("(a p) one -> p (a one)", p=PT),
            )
            for it in range(nt):
                _emit_tile(nc, ld, pool, hpool, amount, mcc, tr, sl_sb, out, it, t_len)
    return nc


_NC = None


def _make_in_maps(inputs):
    amount = inputs["amount"] if isinstance(inputs, dict) else inputs
    if isinstance(inputs, dict):
        amount, mcc_code = inputs["amount"], inputs["mcc_code"]
        tr_type, seq_lens = inputs["tr_type"], inputs["seq_lens"]
    in_maps = []
    for c in range(NCORES):
        rs = slice(c * R, (c + 1) * R)
        in_maps.append(
            {
                "amount": np.ascontiguousarray(amount[rs]),
                "mcc": np.ascontiguousarray(mcc_code[rs]),
                "tr": np.ascontiguousarray(tr_type[rs]),
                "seq": np.ascontiguousarray(seq_lens[rs]).reshape(R, 1),
            }
        )
    return in_maps


def kernel(amount, mcc_code, tr_type, seq_lens):
    global _NC
    if _NC is None:
        _NC = build_nc()
        _NC.finalize()
    in_maps = _make_in_maps(
        {
            "amount": amount,
            "mcc_code": mcc_code,
            "tr_type": tr_type,
            "seq_lens": seq_lens,
        }
    )
    res = run_bass_kernel_spmd(_NC, in_maps, list(range(NCORES))).results
    return np.concatenate([res[c]["out"] for c in range(NCORES)], axis=0)


# revision 16
# speedup vs baseline: 7.3750x; 7.3750x over previous
"""Trainium2 Bass kernel for nn_AggFeatureSeqEncoder (histogram binning).

Per row b: row stats over T, plus per-category (V=256 and V=128)
count / value-sum / value^2-sum histograms, reduced to
count / mean / std features, plus distinct-category counts.

Sharding: pure data parallelism, B=4096 rows split 512/core over 8 cores.
"""

import numpy as np

import concourse.bass as bass
import concourse.bacc as bacc
import concourse.mybir as mybir
from concourse.tile import TileContext
from concourse.bass_utils import run_bass_kernel_spmd

B, T = 4096, 2048
NCORES = 8
R = B // NCORES  # rows per core
PT = 128         # partition tile (rows per SBUF tile)
VM, VT = 256, 128
H = 4 + 3 * VM + 3 * VT + 2  # 1158
EPS = 1e-9

f32 = mybir.dt.float32
bf16 = mybir.dt.bfloat16
i32 = mybir.dt.int32
Alu = mybir.AluOpType
Act = mybir.ActivationFunctionType


def _emit_tile(nc, ld, pool, hpool, amount, mcc, tr, sl_sb, bias_t, out, it, t_len):
    """Emit instructions for one 128-row tile."""
    rows = slice(it * PT, (it + 1) * PT)

    # ---- loads -----------------------------------------------------------
    a = ld.tile([PT, t_len], f32, tag="a")
    nc.gpsimd.dma_start(out=a[:], in_=amount[rows, :])
    mcc_t = ld.tile([PT, t_len], i32, tag="mcc")
    nc.gpsimd.dma_start(out=mcc_t[:], in_=mcc[rows, :])
    tr_t = ld.tile([PT, t_len], i32, tag="tr")
    nc.gpsimd.dma_start(out=tr_t[:], in_=tr[rows, :])

    # ---- val = sign(a) * (exp(|a|) - 1) ---------------------------------
    sgn = pool.tile([PT, t_len], f32, tag="sgn")
    nc.scalar.activation(sgn[:], a[:], Act.Sign)
    ex = pool.tile([PT, t_len], f32, tag="ex")
    nc.scalar.activation(ex[:], a[:], Act.Abs)
    nc.scalar.activation(ex[:], ex[:], Act.Exp)
    val = pool.tile([PT, t_len], f32, tag="val")
    # val = (ex - 1) * sgn
    nc.vector.scalar_tensor_tensor(val[:], ex[:], 1.0, sgn[:], Alu.subtract, Alu.mult)

    # ---- row stats -------------------------------------------------------
    sum_ = hpool.tile([PT, 1], f32, tag="sum")
    sumsq = hpool.tile([PT, 1], f32, tag="sumsq")
    scr = pool.tile([PT, t_len], f32, tag="scr")
    nc.scalar.activation(scr[:], val[:], Act.Copy, accum_out=sum_[:])
    # val2 must be the square of the SAME val used for sv, or the
    # sv2 - sv^2/cnt cancellation breaks for single-element bins.
    val2_f = pool.tile([PT, t_len], f32, tag="val2f")
    nc.scalar.activation(val2_f[:], val[:], Act.Square, accum_out=sumsq[:])

    # bf16 + f32 copies of the codes (exact: values < 256)
    mcc_b = pool.tile([PT, t_len], bf16, tag="mccb")
    nc.gpsimd.tensor_scalar_add(mcc_b[:], mcc_t[:], 0)
    tr_b = pool.tile([PT, t_len], bf16, tag="trb")
    nc.gpsimd.tensor_scalar_add(tr_b[:], tr_t[:], 0)
    mcc_f = pool.tile([PT, t_len], f32, tag="mccf")
    nc.vector.tensor_scalar_add(mcc_f[:], mcc_t[:], 0)
    tr_f = pool.tile([PT, t_len], f32, tag="trf")
    nc.vector.tensor_scalar_add(tr_f[:], tr_t[:], 0)

    out_sb = hpool.tile([PT, H], f32, tag="outsb")

    sl_f = hpool.tile([PT, 1], f32, tag="slf")
    nc.vector.tensor_scalar_add(sl_f[:], sl_sb[:, it : it + 1], 0)
    # mean = sum / (sl + eps)
    tmp0 = hpool.tile([PT, 1], f32, tag="tmp0")
    nc.vector.tensor_scalar_add(tmp0[:], sl_f[:], EPS)
    r1 = hpool.tile([PT, 1], f32, tag="r1")
    nc.vector.reciprocal(r1[:], tmp0[:])
    nc.vector.tensor_tensor(out_sb[:, 2:3], sum_[:], r1[:], Alu.mult)
    # std = sqrt(clip(sumsq - sum^2/(sl+eps), 0) / (clip(sl-1,0)+eps))
    s2 = hpool.tile([PT, 1], f32, tag="s2")
    nc.vector.tensor_tensor(s2[:], sum_[:], sum_[:], Alu.mult)
    nc.vector.tensor_tensor(s2[:], s2[:], r1[:], Alu.mult)
    av = hpool.tile([PT, 1], f32, tag="av")
    nc.vector.tensor_tensor(av[:], sumsq[:], s2[:], Alu.subtract)
    nc.vector.tensor_scalar_max(av[:], av[:], 0.0)
    d = hpool.tile([PT, 1], f32, tag="d")
    nc.vector.tensor_scalar_sub(d[:], sl_f[:], 1.0)
    nc.vector.tensor_scalar_max(d[:], d[:], 0.0)
    nc.vector.tensor_scalar_add(d[:], d[:], EPS)
    nc.vector.reciprocal(d[:], d[:])
    nc.vector.tensor_tensor(av[:], av[:], d[:], Alu.mult)
    nc.scalar.activation(out_sb[:, 3:4], av[:], Act.Sqrt)
    nc.vector.tensor_scalar_add(out_sb[:, 0:1], sl_f[:], 0.0)
    nc.vector.tensor_scalar_add(out_sb[:, 1:2], sum_[:], 0.0)

    # ---- histograms ------------------------------------------------------
    eq = pool.tile([PT, t_len], bf16, tag="eq")
    eq2 = pool.tile([PT, t_len], f32, tag="eq2")
    eq3 = pool.tile([PT, t_len], f32, tag="eq3")

    for name, code_b, code_f, V in (("m", mcc_b, mcc_f, VM), ("t", tr_b, tr_f, VT)):
        cnt = hpool.tile([PT, V], f32, tag="cnt" + name)
        sv = hpool.tile([PT, V], f32, tag="sv" + name)
        sv2 = hpool.tile([PT, V], f32, tag="sv2" + name)
        nc.vector.memset(cnt[:, 0:1], 0.0)
        nc.vector.memset(sv[:, 0:1], 0.0)
        nc.vector.memset(sv2[:, 0:1], 0.0)
        # thermometer counts on ACT: S_v = sum(sign(code - v + 0.5)) so
        # C_v = #(code >= v) = (S_v + T)/2; cnt_v = C_v - C_{v+1}
        thermo = hpool.tile([PT, V + 1], f32, tag="thermo" + name)
        for v in range(1, V):
            nc.scalar.activation(
                eq[:], code_f[:], Act.Sign, bias=bias_t[:, v : v + 1],
                accum_out=thermo[:, v : v + 1],
            )
        nc.vector.tensor_scalar(
            thermo[:, 1:V], thermo[:, 1:V], float(t_len), 0.5, Alu.add, Alu.mult
        )
        nc.vector.memset(thermo[:, V : V + 1], 0.0)
        nc.vector.tensor_tensor(
            cnt[:, 1:V], thermo[:, 1:V], thermo[:, 2 : V + 1], Alu.subtract
        )
        for v in range(1, V):
            fv = float(v)
            nc.vector.scalar_tensor_tensor(
                eq2[:], code_b[:], fv, val[:], Alu.is_equal, Alu.mult,
                accum_out=sv[:, v : v + 1],
            )
            nc.vector.scalar_tensor_tensor(
                eq3[:], code_b[:], fv, val2_f[:], Alu.is_equal, Alu.mult,
                accum_out=sv2[:, v : v + 1],
            )

        # ---- per-category features --------------------------------------
        if name == "m":
            o_cnt, o_mean, o_std, o_dist = 4, 4 + VM, 4 + 2 * VM, H - 2
        else:
            base = 4 + 3 * VM
            o_cnt, o_mean, o_std, o_dist = base, base + VT, base + 2 * VT, H - 1

        ct = hpool.tile([PT, V], f32, tag="ct" + name)
        rc = hpool.tile([PT, V], f32, tag="rc" + name)
        nc.vector.tensor_scalar_add(ct[:], cnt[:], EPS)
        nc.vector.reciprocal(rc[:], ct[:])
        mean_c = out_sb[:, o_mean : o_mean + V]
        nc.vector.tensor_tensor(mean_c, sv[:], rc[:], Alu.mult)
        # av = clip(sv2 - sv*mean, 0); denom = clip(cnt-1,0)+eps
        q = hpool.tile([PT, V], f32, tag="q" + name)
        nc.vector.tensor_tensor(q[:], sv[:], mean_c, Alu.mult)
        nc.vector.tensor_tensor(q[:], sv2[:], q[:], Alu.subtract)
        nc.vector.tensor_scalar_max(q[:], q[:], 0.0)
        nc.vector.tensor_scalar_sub(ct[:], cnt[:], 1.0)
        nc.vector.tensor_scalar_max(ct[:], ct[:], 0.0)
        nc.vector.tensor_scalar_add(ct[:], ct[:], EPS)
        nc.vector.reciprocal(rc[:], ct[:])
        nc.vector.tensor_tensor(q[:], q[:], rc[:], Alu.mult)
        nc.scalar.activation(out_sb[:, o_std : o_std + V], q[:], Act.Sqrt)
        nc.vector.tensor_scalar_add(out_sb[:, o_cnt : o_cnt + V], cnt[:], 0.0)
        # distinct count
        nc.vector.tensor_scalar(
            ct[:], cnt[:], 0.0, None, Alu.is_gt, Alu.add,
            accum_out=out_sb[:, o_dist : o_dist + 1],
        )

    out_stage = hpool.tile([PT, H], f32, tag="outstage")
    nc.scalar.activation(out_stage[:], out_sb[:], Act.Copy)
    nc.gpsimd.dma_start(out=out[rows, :], in_=out_stage[:])


def build_nc(r=R, t_len=T):
    nc = bacc.Bacc()
    amount = nc.declare_dram_parameter("amount", [r, t_len], f32, False)
    mcc = nc.declare_dram_parameter("mcc", [r, t_len], i32, False)
    tr = nc.declare_dram_parameter("tr", [r, t_len], i32, False)
    seq = nc.declare_dram_parameter("seq", [r, 1], i32, False)
    out = nc.declare_dram_parameter("out", [r, H], f32, True)
    with TileContext(nc) as tc:
        with (
            tc.tile_pool(name="ld", bufs=2) as ld,
            tc.tile_pool(name="wk", bufs=1) as pool,
            tc.tile_pool(name="hist", bufs=1) as hpool,
        ):
            nt = r // PT
            sl_sb = hpool.tile([PT, nt], i32, tag="slsb")
            nc.gpsimd.dma_start(
                out=sl_sb[:],
                in_=seq[:].rearrange("(a p) one -> p (a one)", p=PT),
            )
            # bias_t[:, v] = 0.5 - v for the ACT thermometer pass
            bias_i = hpool.tile([PT, VM], i32, tag="biasi")
            nc.gpsimd.iota(bias_i[:], [[-1, VM]], base=0, channel_multiplier=0)
            bias_t = hpool.tile([PT, VM], f32, tag="biast")
            nc.vector.tensor_scalar_add(bias_t[:], bias_i[:], 0.5)
            for it in range(nt):
                _emit_tile(
                    nc, ld, pool, hpool, amount, mcc, tr, sl_sb, bias_t, out,
                    it, t_len,
                )
    return nc


_NC = None


def _make_in_maps(inputs):
    amount = inputs["amount"] if isinstance(inputs, dict) else inputs
    if isinstance(inputs, dict):
        amount, mcc_code = inputs["amount"], inputs["mcc_code"]
        tr_type, seq_lens = inputs["tr_type"], inputs["seq_lens"]
    in_maps = []
    for c in range(NCORES):
        rs = slice(c * R, (c + 1) * R)
        in_maps.append(
            {
                "amount": np.ascontiguousarray(amount[rs]),
                "mcc": np.ascontiguousarray(mcc_code[rs]),
                "tr": np.ascontiguousarray(tr_type[rs]),
                "seq": np.ascontiguousarray(seq_lens[rs]).reshape(R, 1),
            }
        )
    return in_maps


def kernel(amount, mcc_code, tr_type, seq_lens):
    global _NC
    if _NC is None:
        _NC = build_nc()
        _NC.finalize()
    in_maps = _make_in_maps(
        {
            "amount": amount,
            "mcc_code": mcc_code,
            "tr_type": tr_type,
            "seq_lens": seq_lens,
        }
    )
    res = run_bass_kernel_spmd(_NC, in_maps, list(range(NCORES))).results
    return np.concatenate([res[c]["out"] for c in range(NCORES)], axis=0)
